# revision 83
# baseline (speedup 1.0000x reference)
"""2-layer GCN (2 edge types + self loop) on 8 TRN2 NeuronCores.

Sharding: nodes split contiguously across 8 cores (6250/core, padded to
6272 = 49 windows x 128 rows); edge lists partitioned by destination
owner, sorted by (dst window, src half); [128,128] weights replicated.

v2: aggregate-then-transform. Because GraphConv is linear,
  agg(A, h @ W) == agg(A, h) @ W,
so the table that must be shared each layer is h itself (shared by
BOTH edge types) instead of two per-etype m tables. Per dst window, the
two per-etype aggregates are built by selection-matrix matmuls
accumulating in PSUM (1/deg folded into the selection values), copied
to SBUF, then transformed by W_a/W_b plus the self-loop matmul in a
second PSUM group, with bias+ReLU fused into the PSUM->SBUF copy on the
scalar engine.

v3/v4: pushing the same linearity one level deeper,
  Ahat(x @ W_proj) == (Ahat x) @ W_proj,
so the layer-1 gather table is node-major x itself -- an input, uploaded
directly; no on-device table build and no layer-1 collective at all.
W_proj is folded into the layer-1 transform weights on the host
(W_proj @ W1_x in f32), and b_proj's aggregated contribution into the
layer-1 bias (exact whenever b_proj == 0 or no zero-degree nodes).
Only layer 2's table (h1, node-sharded) is built on device and
AllGathered. A host-side within-core permutation (_layout) shapes
per-(window, etype, src-half) edge counts toward just-under-multiples
of 128, cutting chunk padding to ~4%.
"""
import sys
import hashlib

sys.path.insert(0, "/opt/trn_rl_repo")

import numpy as np
import ml_dtypes

import concourse.bass as bass  # noqa: F401
import concourse.bacc as bacc
import concourse.mybir as mybir
import concourse.tile as tile
from concourse.bass_utils import run_bass_kernel_spmd

N = 50000
D = 128
NCORES = 8
LOCAL = 6250          # real rows per core
SHARD = 6272          # padded rows per core (49 windows of 128)
NW = 49               # dst windows per core
VN = SHARD * NCORES   # 50176 virtual node rows
HALFR = VN // 2       # 25088, int16-addressable half of the gathered table
WGROUPS = [list(range(i, min(i + 5, NW))) for i in range(0, NW, 5)]  # gather groups

F32 = mybir.dt.float32
BF16 = mybir.dt.bfloat16
I16 = mybir.dt.int16
BF = ml_dtypes.bfloat16

_compiled = {}
_prep_cache = {}

_layout_cache = {}


def _layout(src_a, dst_a, src_b, dst_b):
    """Within-core node->slot permutation that shapes per-(window, etype,
    src-half) edge counts toward just-under-multiples of 128, cutting chunk
    padding. Node->core stays contiguous (it defines the int16 half split),
    so the per-node components are fixed and the packing decouples per core.
    Returns vmap[node] -> virtual row (core*SHARD + slot)."""
    key = hashlib.sha1(src_a.tobytes() + dst_a.tobytes()
                       + src_b.tobytes() + dst_b.tobytes()).digest()
    if key in _layout_cache:
        return _layout_cache[key]
    core_of = np.arange(N, dtype=np.int64) // LOCAL
    comps = np.zeros((N, 4), np.int64)
    for ci, (src, dst) in enumerate(((src_a, dst_a), (src_b, dst_b))):
        hh = (core_of[src] >= NCORES // 2).astype(np.int64)
        np.add.at(comps, (dst, 2 * ci + hh), 1)
    # K template: a few heavy windows (K=9), rest light (K=8), same profile
    # for every core and component; bump if capacity can't cover the worst
    # per-core component total
    Ktmpl = np.where(np.arange(NW) % 4 == 1, 9, 8).astype(np.int64)
    worst = max(int(comps[r * LOCAL:(r + 1) * LOCAL].sum(axis=0).max())
                for r in range(NCORES))
    margin = 24
    while int((Ktmpl * 128 - margin).sum()) < worst:
        Ktmpl[int(np.argmin(Ktmpl))] += 1
    cap0 = Ktmpl * 128 - margin
    slot_of = np.zeros(N, np.int64)
    for r in range(NCORES):
        g = comps[r * LOCAL:(r + 1) * LOCAL]
        order = np.argsort(-g.sum(axis=1), kind="stable")
        cap = np.tile(cap0[:, None], (1, 4))
        slots_left = np.full(NW, 128, np.int64)
        slots_left[NW - 1] = LOCAL - 128 * (NW - 1)
        win_of = np.empty(LOCAL, np.int64)
        for d in order:
            head = (cap - g[d]).min(axis=1).astype(np.float64)
            head[slots_left <= 0] = -np.inf
            wsel = int(np.argmax(head))
            win_of[d] = wsel
            cap[wsel] -= g[d]
            slots_left[wsel] -= 1
        cnt = np.zeros(NW, np.int64)
        slot = np.empty(LOCAL, np.int64)
        for d in range(LOCAL):
            w = win_of[d]
            slot[d] = w * 128 + cnt[w]
            cnt[w] += 1
        slot_of[r * LOCAL:(r + 1) * LOCAL] = slot
    vmap = core_of * SHARD + slot_of
    _layout_cache[key] = vmap
    return vmap



def _prep_etype(src, dst, vmap):
    """Per-edge-type host prep. Returns (K[w][h] chunk table,
    per-core wrapped idx / dst_rel / v arrays laid out in canonical
    chunk order: for g, for h, for w in g, for k)."""
    key = hashlib.sha1(src.tobytes() + dst.tobytes() + vmap.tobytes()).digest()
    if key in _prep_cache:
        return _prep_cache[key]
    deg = np.bincount(dst, minlength=N).astype(np.float32)
    v_edge = (1.0 / np.maximum(deg, 1.0))[dst].astype(np.float32)
    dvid = vmap[dst]
    r = dvid // SHARD
    w = (dvid % SHARD) // 128
    wloc = (dvid % 128).astype(np.float32)
    svid = vmap[src]
    h = svid // HALFR
    i16 = (svid % HALFR).astype(np.int16)

    key2 = (r * NW + w) * 2 + h
    order = np.argsort(key2, kind="stable")
    counts = np.bincount(key2, minlength=NCORES * NW * 2).reshape(NCORES, NW, 2)
    flat = counts.reshape(-1)
    fs = np.concatenate([[0], np.cumsum(flat)[:-1]])
    starts = fs.reshape(NCORES, NW, 2)

    K = np.maximum(1, (counts.max(axis=0) + 127) // 128)  # [NW, 2]

    # canonical chunk order
    chunk_off = {}
    nch = 0
    for g in WGROUPS:
        for hh in (0, 1):
            for ww in g:
                chunk_off[(ww, hh)] = nch
                nch += int(K[ww, hh])

    i16_s = i16[order]
    wloc_s = wloc[order]
    v_s = v_edge[order]

    idx_all, dst_all, v_all = [], [], []
    for rr in range(NCORES):
        idx_pad = np.zeros(nch * 128, np.int16)
        dst_pad = np.full(nch * 128, -1.0, np.float32)
        v_pad = np.ones(nch * 128, np.float32)
        for ww in range(NW):
            for hh in (0, 1):
                s0 = starts[rr, ww, hh]
                c = counts[rr, ww, hh]
                o = chunk_off[(ww, hh)] * 128
                idx_pad[o : o + c] = i16_s[s0 : s0 + c]
                dst_pad[o : o + c] = wloc_s[s0 : s0 + c]
                v_pad[o : o + c] = v_s[s0 : s0 + c]
        wrapped = np.tile(idx_pad.reshape(-1, 16).T, (8, 1))  # [128, nch*8]
        idx_all.append(np.ascontiguousarray(wrapped))
        dst_all.append(np.ascontiguousarray(dst_pad.reshape(nch, 128).T))
        v_all.append(np.ascontiguousarray(v_pad.reshape(nch, 128).T))
    res = (K, chunk_off, nch, idx_all, dst_all, v_all)
    _prep_cache[key] = res
    return res


def _build(K_a, off_a, nch_a, K_b, off_b, nch_b):
    nc = bacc.Bacc("TRN2", target_bir_lowering=False, debug=False)

    xT_in = nc.dram_tensor("xT", [128, SHARD], BF16, kind="ExternalInput")
    xnode_in = nc.dram_tensor("xnode", [VN, 128], BF16, kind="ExternalInput")
    w_names = ["W_proj", "W1_a", "W1_b", "loop1", "W2_a", "W2_b", "loop2"]
    w_in = {n: nc.dram_tensor(n, [128, 128], BF16, kind="ExternalInput") for n in w_names}
    b_names = ["bias_proj", "bias1", "bias2"]
    b_in = {n: nc.dram_tensor(n, [128, 1], F32, kind="ExternalInput") for n in b_names}
    iota_in = nc.dram_tensor("iota", [128, 128], BF16, kind="ExternalInput")
    ident_in = nc.dram_tensor("ident", [128, 128], BF16, kind="ExternalInput")
    idx_in = [
        nc.dram_tensor("idx_a", [128, nch_a * 8], I16, kind="ExternalInput"),
        nc.dram_tensor("idx_b", [128, nch_b * 8], I16, kind="ExternalInput"),
    ]
    dst_in = [
        nc.dram_tensor("dst_a", [128, nch_a], F32, kind="ExternalInput"),
        nc.dram_tensor("dst_b", [128, nch_b], F32, kind="ExternalInput"),
    ]
    v_in = [
        nc.dram_tensor("v_a", [128, nch_a], F32, kind="ExternalInput"),
        nc.dram_tensor("v_b", [128, nch_b], F32, kind="ExternalInput"),
    ]
    out = nc.dram_tensor("out", [128, SHARD], F32, kind="ExternalOutput")

    Ks = [K_a, K_b]
    offs = [off_a, off_b]

    with tile.TileContext(nc) as tc:
        with (
            tc.tile_pool(name="sbuf", bufs=1) as sb,
            tc.tile_pool(name="psum", bufs=1, space="PSUM") as ps,
            tc.tile_pool(name="dram", bufs=1, space="DRAM") as dr,
        ):
            # ---- constants / persistent buffers
            w_sb = {}
            for n in w_names:
                w_sb[n] = sb.tile([128, 128], BF16, tag=f"w_{n}", name=f"w_{n}")
                nc.sync.dma_start(out=w_sb[n][:], in_=w_in[n][:])
            b_sb = {}
            for n in b_names:
                b_sb[n] = sb.tile([128, 1], F32, tag=f"b_{n}", name=f"b_{n}")
                nc.sync.dma_start(out=b_sb[n][:], in_=b_in[n][:])
            iota_sb = sb.tile([128, 128], BF16, tag="iota")
            nc.sync.dma_start(out=iota_sb[:], in_=iota_in[:])
            ident_sb = sb.tile([128, 128], BF16, tag="ident")
            nc.sync.dma_start(out=ident_sb[:], in_=ident_in[:])

            xT = sb.tile([128, SHARD], BF16, tag="hstate", bufs=2)
            nc.sync.dma_start(out=xT[:], in_=xT_in[:])
            dst_sb = []
            v_sb = []
            for t in (0, 1):
                d = sb.tile([128, [nch_a, nch_b][t]], F32, tag=f"dst{t}", name=f"dst{t}")
                nc.sync.dma_start(out=d[:], in_=dst_in[t][:])
                dst_sb.append(d)
                vv = sb.tile([128, [nch_a, nch_b][t]], F32, tag=f"v{t}", name=f"v{t}")
                nc.sync.dma_start(out=vv[:], in_=v_in[t][:])
                v_sb.append(vv)

            hT = sb.tile([128, SHARD], BF16, tag="hstate", bufs=2)
            h1T = sb.tile([128, SHARD], BF16, tag="hstate", bufs=2)

            # layer-1 gather table: by linearity Ahat(x@W_proj) ==
            # (Ahat x)@W_proj, so the table is node-major x itself (an
            # input); W_proj is folded into the layer-1 transform weights
            # on the host. Only layer 2's table (h1) needs building+AllGather.
            table0 = xnode_in
            ag_in = dr.tile([SHARD, 128], BF16, tag="agi1", name="agi1")
            ag_out = dr.tile([VN, 128], BF16, tag="ago1", name="ago1", addr_space="Shared")

            def col_chunks(total, step):
                o = 0
                while o < total:
                    yield o, min(step, total - o)
                    o += step

            # ---- phase P: hT = (x @ W_proj + b_proj)^T  (feature-major,
            # local; feeds the self-loop terms)
            for o, n in col_chunks(SHARD, 512):
                p = ps.tile([128, 512], F32, tag="pdense", bufs=2)
                nc.tensor.matmul(p[:, :n], lhsT=w_sb["W_proj"][:], rhs=xT[:, o : o + n],
                                 start=True, stop=True)
                nc.vector.tensor_scalar_add(hT[:, o : o + n], p[:, :n], b_sb["bias_proj"][:, :1])

            # ---- layers
            for l in (0, 1):
                src_hT = hT if l == 0 else h1T
                wa, wb, wl = (("W1_a", "W1_b", "loop1") if l == 0 else ("W2_a", "W2_b", "loop2"))
                bias = b_sb["bias1"] if l == 0 else b_sb["bias2"]
                table = table0 if l == 0 else ag_out

                def emit_gather(t, hh, wins, gb):
                    nslab = sum(int(Ks[t][w, hh]) for w in wins)
                    ci0 = offs[t][(wins[0], hh)]
                    gidx = sb.tile([128, nslab * 8], I16, tag=f"gi{t}{hh}",
                                   name=f"gi{t}{hh}", bufs=2)
                    nc.sync.dma_start(out=gidx[:], in_=idx_in[t][:, ci0 * 8 : (ci0 + nslab) * 8])
                    gbuf = sb.tile([128, nslab, 128], BF16, tag=f"gb{t}{hh}",
                                   name=f"gb{t}{hh}", bufs=3 if hh == 0 else 2)
                    nc.gpsimd.dma_gather(
                        gbuf[:],
                        table[hh * HALFR : (hh + 1) * HALFR, :],
                        gidx[:],
                        nslab * 128,
                        nslab * 128,
                        128,
                        single_packet=False,
                    )
                    gb[(t, hh)] = (gbuf, ci0)

                gbs = [dict() for _ in WGROUPS]
                if l == 0:
                    # table0's half-0 finishes writing well before half-1:
                    # front-load the first two groups' half-0 gathers so the
                    # in-order Pool queue isn't blocked by a half-1 wait
                    for g in (0, 1):
                        for t in (0, 1):
                            emit_gather(t, 0, WGROUPS[g], gbs[g])
                    for g in (0, 1):
                        for t in (0, 1):
                            emit_gather(t, 1, WGROUPS[g], gbs[g])
                for g, wins in enumerate(WGROUPS):
                    gb = gbs[g]
                    if not gb:
                        for t in (0, 1):
                            for hh in (0, 1):
                                emit_gather(t, hh, wins, gb)
                    for w in wins:
                        agg_sb = []
                        for t in (0, 1):
                            nk = int(Ks[t][w, 0]) + int(Ks[t][w, 1])
                            pagg = ps.tile([128, 128], F32, tag="pagg", bufs=4)
                            ki = 0
                            for hh in (0, 1):
                                gbuf, ci0 = gb[(t, hh)]
                                slab0 = offs[t][(w, hh)] - ci0
                                for k in range(int(Ks[t][w, hh])):
                                    ci = offs[t][(w, hh)] + k
                                    s = sb.tile([128, 128], BF16, tag="s", bufs=72)
                                    nc.vector.tensor_scalar(
                                        out=s[:],
                                        in0=iota_sb[:],
                                        scalar1=dst_sb[t][:, ci : ci + 1],
                                        scalar2=v_sb[t][:, ci : ci + 1],
                                        op0=mybir.AluOpType.is_equal,
                                        op1=mybir.AluOpType.mult,
                                    )
                                    nc.tensor.matmul(pagg[:], lhsT=gbuf[:, slab0 + k, :], rhs=s[:],
                                                     start=(ki == 0), stop=(ki == nk - 1))
                                    ki += 1
                            a = sb.tile([128, 128], BF16, tag=f"agg{t}", bufs=3)
                            nc.scalar.activation(out=a[:], in_=pagg[:],
                                                 func=mybir.ActivationFunctionType.Copy)
                            agg_sb.append(a)
                        pf = ps.tile([128, 128], F32, tag="pf", bufs=1)
                        nc.tensor.matmul(pf[:], lhsT=w_sb[wa][:], rhs=agg_sb[0][:],
                                         start=True, stop=False)
                        nc.tensor.matmul(pf[:], lhsT=w_sb[wb][:], rhs=agg_sb[1][:],
                                         start=False, stop=False)
                        nc.tensor.matmul(pf[:], lhsT=w_sb[wl][:],
                                         rhs=src_hT[:, w * 128 : (w + 1) * 128],
                                         start=False, stop=True)
                        if l == 1:
                            o2 = sb.tile([128, 128], F32, tag="o2", bufs=3)
                            nc.scalar.activation(out=o2[:], in_=pf[:],
                                                 func=mybir.ActivationFunctionType.Relu,
                                                 bias=bias[:, :1], scale=1.0)
                            nc.sync.dma_start(out=out[:, w * 128 : (w + 1) * 128],
                                              in_=o2[:])
                        if l == 0:
                            nc.scalar.activation(out=h1T[:, w * 128 : (w + 1) * 128], in_=pf[:],
                                                 func=mybir.ActivationFunctionType.Relu,
                                                 bias=bias[:, :1], scale=1.0)
                            pt = ps.tile([128, 128], BF16, tag="ptr", bufs=1)
                            nc.tensor.transpose(pt[:], h1T[:, w * 128 : (w + 1) * 128],
                                                ident_sb[:])
                            hn = sb.tile([128, 128], BF16, tag="hn", bufs=2)
                            nc.scalar.activation(out=hn[:], in_=pt[:],
                                                 func=mybir.ActivationFunctionType.Copy)
                            nc.sync.dma_start(out=ag_in[w * 128 : (w + 1) * 128, :], in_=hn[:])
                if l == 0:
                    nc.gpsimd.collective_compute(
                        "AllGather",
                        mybir.AluOpType.bypass,
                        replica_groups=[list(range(NCORES))],
                        ins=[ag_in.opt()],
                        outs=[ag_out.opt()],
                    )

    nc.compile()
    return nc


def prepare(**inputs):
    """Build (or reuse) the compiled Bass module and the per-core input maps."""
    x = np.asarray(inputs["x"], np.float32)
    vmap = _layout(np.asarray(inputs["src_a"]), np.asarray(inputs["dst_a"]),
                   np.asarray(inputs["src_b"]), np.asarray(inputs["dst_b"]))
    prep_a = _prep_etype(np.asarray(inputs["src_a"]), np.asarray(inputs["dst_a"]), vmap)
    prep_b = _prep_etype(np.asarray(inputs["src_b"]), np.asarray(inputs["dst_b"]), vmap)
    K_a, off_a, nch_a, idx_a, dst_a, v_a = prep_a
    K_b, off_b, nch_b, idx_b, dst_b, v_b = prep_b

    key = (nch_a, nch_b, K_a.tobytes(), K_b.tobytes())
    if key not in _compiled:
        _compiled[key] = _build(K_a, off_a, nch_a, K_b, off_b, nch_b)
    nc = _compiled[key]

    x_pad = np.zeros((NCORES, SHARD, D), np.float32)
    x_pad.reshape(VN, D)[vmap] = x
    xnode = np.ascontiguousarray(x_pad.reshape(VN, D)).astype(BF)

    Wp_f = np.asarray(inputs["W_proj"], np.float32)
    # layer 1 aggregates raw x; W_proj is folded into its transform weights
    weights = {
        "W_proj": inputs["W_proj"],
        "W1_a": Wp_f @ np.asarray(inputs["W1_a"], np.float32),
        "W1_b": Wp_f @ np.asarray(inputs["W1_b"], np.float32),
        "loop1": inputs["loop1"], "W2_a": inputs["W2_a"], "W2_b": inputs["W2_b"],
        "loop2": inputs["loop2"],
    }
    w_np = {k: np.asarray(v, np.float32).astype(BF) for k, v in weights.items()}
    b_proj = np.asarray(inputs["b_proj"], np.float32)
    W1_a = np.asarray(inputs["W1_a"], np.float32)
    W1_b = np.asarray(inputs["W1_b"], np.float32)
    # table0 omits b_proj; its layer-1 contribution (b_proj @ W1_x per dst
    # row with in-degree > 0) is folded into bias1. Exact when b_proj == 0
    # (the given spec) or when no destination has zero in-degree.
    bias1_eff = (np.asarray(inputs["b1_a"], np.float32)
                 + np.asarray(inputs["b1_b"], np.float32)
                 + b_proj @ W1_a + b_proj @ W1_b)
    biases = {
        "bias_proj": b_proj.reshape(128, 1),
        "bias1": bias1_eff.reshape(128, 1),
        "bias2": (np.asarray(inputs["b2_a"], np.float32)
                  + np.asarray(inputs["b2_b"], np.float32)).reshape(128, 1),
    }
    iota = np.tile(np.arange(128, dtype=np.float32).astype(BF), (128, 1))
    ident = np.eye(128, dtype=np.float32).astype(BF)

    in_maps = []
    for c in range(NCORES):
        m = {
            "xT": np.ascontiguousarray(x_pad[c].T).astype(BF),
            "xnode": xnode,
            "iota": iota,
            "ident": ident,
            "idx_a": idx_a[c], "idx_b": idx_b[c],
            "dst_a": dst_a[c], "dst_b": dst_b[c],
            "v_a": v_a[c], "v_b": v_b[c],
        }
        m.update(w_np)
        m.update(biases)
        in_maps.append(m)
    return nc, in_maps


def kernel(**inputs):
    nc, in_maps = prepare(**inputs)
    res = run_bass_kernel_spmd(nc, in_maps, core_ids=list(range(NCORES)))
    globals()["_last_result"] = res
    vmap = _layout(np.asarray(inputs["src_a"]), np.asarray(inputs["dst_a"]),
                   np.asarray(inputs["src_b"]), np.asarray(inputs["dst_b"]))
    full_virt = np.concatenate(
        [np.asarray(res.results[c]["out"]).T for c in range(NCORES)], axis=0
    )
    return full_virt[vmap].astype(np.float32)


# revision 84
# speedup vs baseline: 1.0088x; 1.0088x over previous
"""2-layer GCN (2 edge types + self loop) on 8 TRN2 NeuronCores.

Sharding: nodes split contiguously across 8 cores (6250/core, padded to
6272 = 49 windows x 128 rows); edge lists partitioned by destination
owner, sorted by (dst window, src half); [128,128] weights replicated.

v2: aggregate-then-transform. Because GraphConv is linear,
  agg(A, h @ W) == agg(A, h) @ W,
so the table that must be shared each layer is h itself (shared by
BOTH edge types) instead of two per-etype m tables. Per dst window, the
two per-etype aggregates are built by selection-matrix matmuls
accumulating in PSUM (1/deg folded into the selection values), copied
to SBUF, then transformed by W_a/W_b plus the self-loop matmul in a
second PSUM group, with bias+ReLU fused into the PSUM->SBUF copy on the
scalar engine.

v3/v4: pushing the same linearity one level deeper,
  Ahat(x @ W_proj) == (Ahat x) @ W_proj,
so the layer-1 gather table is node-major x itself -- an input, uploaded
directly; no on-device table build and no layer-1 collective at all.
W_proj is folded into the layer-1 transform weights on the host
(W_proj @ W1_x in f32), and b_proj's aggregated contribution into the
layer-1 bias (exact whenever b_proj == 0 or no zero-degree nodes).
Only layer 2's table (h1, node-sharded) is built on device and
AllGathered. A host-side within-core permutation (_layout) shapes
per-(window, etype, src-half) edge counts toward just-under-multiples
of 128, cutting chunk padding to ~4%.
"""
import sys
import hashlib

sys.path.insert(0, "/opt/trn_rl_repo")

import numpy as np
import ml_dtypes

import concourse.bass as bass  # noqa: F401
import concourse.bacc as bacc
import concourse.mybir as mybir
import concourse.tile as tile
from concourse.bass_utils import run_bass_kernel_spmd

N = 50000
D = 128
NCORES = 8
LOCAL = 6250          # real rows per core
SHARD = 6272          # padded rows per core (49 windows of 128)
NW = 49               # dst windows per core
VN = SHARD * NCORES   # 50176 virtual node rows
HALFR = VN // 2       # 25088, int16-addressable half of the gathered table
WGROUPS = [list(range(i, min(i + 5, NW))) for i in range(0, NW, 5)]  # gather groups

F32 = mybir.dt.float32
BF16 = mybir.dt.bfloat16
I16 = mybir.dt.int16
BF = ml_dtypes.bfloat16

_compiled = {}
_prep_cache = {}

_layout_cache = {}


def _layout(src_a, dst_a, src_b, dst_b):
    """Within-core node->slot permutation that shapes per-(window, etype,
    src-half) edge counts toward just-under-multiples of 128, cutting chunk
    padding. Node->core stays contiguous (it defines the int16 half split),
    so the per-node components are fixed and the packing decouples per core.
    Returns vmap[node] -> virtual row (core*SHARD + slot)."""
    key = hashlib.sha1(src_a.tobytes() + dst_a.tobytes()
                       + src_b.tobytes() + dst_b.tobytes()).digest()
    if key in _layout_cache:
        return _layout_cache[key]
    core_of = np.arange(N, dtype=np.int64) // LOCAL
    comps = np.zeros((N, 4), np.int64)
    for ci, (src, dst) in enumerate(((src_a, dst_a), (src_b, dst_b))):
        hh = (core_of[src] >= NCORES // 2).astype(np.int64)
        np.add.at(comps, (dst, 2 * ci + hh), 1)
    # K template: a few heavy windows (K=9), rest light (K=8), same profile
    # for every core and component; bump if capacity can't cover the worst
    # per-core component total
    Ktmpl = np.where(np.arange(NW) % 4 == 1, 9, 8).astype(np.int64)
    worst = max(int(comps[r * LOCAL:(r + 1) * LOCAL].sum(axis=0).max())
                for r in range(NCORES))
    margin = 24
    while int((Ktmpl * 128 - margin).sum()) < worst:
        Ktmpl[int(np.argmin(Ktmpl))] += 1
    cap0 = Ktmpl * 128 - margin
    slot_of = np.zeros(N, np.int64)
    for r in range(NCORES):
        g = comps[r * LOCAL:(r + 1) * LOCAL]
        order = np.argsort(-g.sum(axis=1), kind="stable")
        cap = np.tile(cap0[:, None], (1, 4))
        slots_left = np.full(NW, 128, np.int64)
        slots_left[NW - 1] = LOCAL - 128 * (NW - 1)
        win_of = np.empty(LOCAL, np.int64)
        for d in order:
            head = (cap - g[d]).min(axis=1).astype(np.float64)
            head[slots_left <= 0] = -np.inf
            wsel = int(np.argmax(head))
            win_of[d] = wsel
            cap[wsel] -= g[d]
            slots_left[wsel] -= 1
        cnt = np.zeros(NW, np.int64)
        slot = np.empty(LOCAL, np.int64)
        for d in range(LOCAL):
            w = win_of[d]
            slot[d] = w * 128 + cnt[w]
            cnt[w] += 1
        slot_of[r * LOCAL:(r + 1) * LOCAL] = slot
    vmap = core_of * SHARD + slot_of
    _layout_cache[key] = vmap
    return vmap



def _prep_etype(src, dst, vmap):
    """Per-edge-type host prep. Returns (K[w][h] chunk table,
    per-core wrapped idx / dst_rel / v arrays laid out in canonical
    chunk order: for g, for h, for w in g, for k)."""
    key = hashlib.sha1(src.tobytes() + dst.tobytes() + vmap.tobytes()).digest()
    if key in _prep_cache:
        return _prep_cache[key]
    deg = np.bincount(dst, minlength=N).astype(np.float32)
    v_edge = (1.0 / np.maximum(deg, 1.0))[dst].astype(np.float32)
    dvid = vmap[dst]
    r = dvid // SHARD
    w = (dvid % SHARD) // 128
    wloc = (dvid % 128).astype(np.float32)
    svid = vmap[src]
    h = svid // HALFR
    i16 = (svid % HALFR).astype(np.int16)

    key2 = (r * NW + w) * 2 + h
    order = np.argsort(key2, kind="stable")
    counts = np.bincount(key2, minlength=NCORES * NW * 2).reshape(NCORES, NW, 2)
    flat = counts.reshape(-1)
    fs = np.concatenate([[0], np.cumsum(flat)[:-1]])
    starts = fs.reshape(NCORES, NW, 2)

    K = np.maximum(1, (counts.max(axis=0) + 127) // 128)  # [NW, 2]

    # canonical chunk order
    chunk_off = {}
    nch = 0
    for g in WGROUPS:
        for hh in (0, 1):
            for ww in g:
                chunk_off[(ww, hh)] = nch
                nch += int(K[ww, hh])

    i16_s = i16[order]
    wloc_s = wloc[order]
    v_s = v_edge[order]

    idx_all, dst_all, v_all = [], [], []
    for rr in range(NCORES):
        idx_pad = np.zeros(nch * 128, np.int16)
        dst_pad = np.full(nch * 128, -1.0, np.float32)
        v_pad = np.ones(nch * 128, np.float32)
        for ww in range(NW):
            for hh in (0, 1):
                s0 = starts[rr, ww, hh]
                c = counts[rr, ww, hh]
                o = chunk_off[(ww, hh)] * 128
                idx_pad[o : o + c] = i16_s[s0 : s0 + c]
                dst_pad[o : o + c] = wloc_s[s0 : s0 + c]
                v_pad[o : o + c] = v_s[s0 : s0 + c]
        wrapped = np.tile(idx_pad.reshape(-1, 16).T, (8, 1))  # [128, nch*8]
        idx_all.append(np.ascontiguousarray(wrapped))
        dst_all.append(np.ascontiguousarray(dst_pad.reshape(nch, 128).T))
        v_all.append(np.ascontiguousarray(v_pad.reshape(nch, 128).T))
    res = (K, chunk_off, nch, idx_all, dst_all, v_all)
    _prep_cache[key] = res
    return res


def _build(K_a, off_a, nch_a, K_b, off_b, nch_b):
    nc = bacc.Bacc("TRN2", target_bir_lowering=False, debug=False)

    xT_in = nc.dram_tensor("xT", [128, SHARD], BF16, kind="ExternalInput")
    xnode_in = nc.dram_tensor("xnode", [VN, 128], BF16, kind="ExternalInput")
    w_names = ["W_proj", "W1_a", "W1_b", "loop1", "W2_a", "W2_b", "loop2"]
    w_in = {n: nc.dram_tensor(n, [128, 128], BF16, kind="ExternalInput") for n in w_names}
    b_names = ["bias_proj", "bias1", "bias2"]
    b_in = {n: nc.dram_tensor(n, [128, 1], F32, kind="ExternalInput") for n in b_names}
    iota_in = nc.dram_tensor("iota", [128, 128], BF16, kind="ExternalInput")
    ident_in = nc.dram_tensor("ident", [128, 128], BF16, kind="ExternalInput")
    idx_in = [
        nc.dram_tensor("idx_a", [128, nch_a * 8], I16, kind="ExternalInput"),
        nc.dram_tensor("idx_b", [128, nch_b * 8], I16, kind="ExternalInput"),
    ]
    dst_in = [
        nc.dram_tensor("dst_a", [128, nch_a], F32, kind="ExternalInput"),
        nc.dram_tensor("dst_b", [128, nch_b], F32, kind="ExternalInput"),
    ]
    v_in = [
        nc.dram_tensor("v_a", [128, nch_a], F32, kind="ExternalInput"),
        nc.dram_tensor("v_b", [128, nch_b], F32, kind="ExternalInput"),
    ]
    out = nc.dram_tensor("out", [128, SHARD], F32, kind="ExternalOutput")

    Ks = [K_a, K_b]
    offs = [off_a, off_b]

    with tile.TileContext(nc) as tc:
        with (
            tc.tile_pool(name="sbuf", bufs=1) as sb,
            tc.tile_pool(name="psum", bufs=1, space="PSUM") as ps,
            tc.tile_pool(name="dram", bufs=1, space="DRAM") as dr,
        ):
            # ---- constants / persistent buffers
            w_sb = {}
            for n in w_names:
                w_sb[n] = sb.tile([128, 128], BF16, tag=f"w_{n}", name=f"w_{n}")
                nc.sync.dma_start(out=w_sb[n][:], in_=w_in[n][:])
            b_sb = {}
            for n in b_names:
                b_sb[n] = sb.tile([128, 1], F32, tag=f"b_{n}", name=f"b_{n}")
                nc.sync.dma_start(out=b_sb[n][:], in_=b_in[n][:])
            iota_sb = sb.tile([128, 128], BF16, tag="iota")
            nc.sync.dma_start(out=iota_sb[:], in_=iota_in[:])
            ident_sb = sb.tile([128, 128], BF16, tag="ident")
            nc.sync.dma_start(out=ident_sb[:], in_=ident_in[:])

            # xT / dst / v go through the otherwise-idle ACT queue so the
            # SP queue reaches the first gather's index loads immediately
            xT = sb.tile([128, SHARD], BF16, tag="hstate", bufs=2)
            nc.scalar.dma_start(out=xT[:], in_=xT_in[:])
            dst_sb = []
            v_sb = []
            for t in (0, 1):
                d = sb.tile([128, [nch_a, nch_b][t]], F32, tag=f"dst{t}", name=f"dst{t}")
                nc.scalar.dma_start(out=d[:], in_=dst_in[t][:])
                dst_sb.append(d)
                vv = sb.tile([128, [nch_a, nch_b][t]], F32, tag=f"v{t}", name=f"v{t}")
                nc.scalar.dma_start(out=vv[:], in_=v_in[t][:])
                v_sb.append(vv)

            hT = sb.tile([128, SHARD], BF16, tag="hstate", bufs=2)
            h1T = sb.tile([128, SHARD], BF16, tag="hstate", bufs=2)

            # layer-1 gather table: by linearity Ahat(x@W_proj) ==
            # (Ahat x)@W_proj, so the table is node-major x itself (an
            # input); W_proj is folded into the layer-1 transform weights
            # on the host. Only layer 2's table (h1) needs building+AllGather.
            table0 = xnode_in
            ag_in = dr.tile([SHARD, 128], BF16, tag="agi1", name="agi1")
            ag_out = dr.tile([VN, 128], BF16, tag="ago1", name="ago1", addr_space="Shared")

            def col_chunks(total, step):
                o = 0
                while o < total:
                    yield o, min(step, total - o)
                    o += step

            # ---- phase P: hT = (x @ W_proj + b_proj)^T  (feature-major,
            # local; feeds the self-loop terms)
            for o, n in col_chunks(SHARD, 512):
                p = ps.tile([128, 512], F32, tag="pdense", bufs=2)
                nc.tensor.matmul(p[:, :n], lhsT=w_sb["W_proj"][:], rhs=xT[:, o : o + n],
                                 start=True, stop=True)
                nc.vector.tensor_scalar_add(hT[:, o : o + n], p[:, :n], b_sb["bias_proj"][:, :1])

            # ---- layers
            for l in (0, 1):
                src_hT = hT if l == 0 else h1T
                wa, wb, wl = (("W1_a", "W1_b", "loop1") if l == 0 else ("W2_a", "W2_b", "loop2"))
                bias = b_sb["bias1"] if l == 0 else b_sb["bias2"]
                table = table0 if l == 0 else ag_out

                def emit_gather(t, hh, wins, gb):
                    nslab = sum(int(Ks[t][w, hh]) for w in wins)
                    ci0 = offs[t][(wins[0], hh)]
                    gidx = sb.tile([128, nslab * 8], I16, tag=f"gi{t}{hh}",
                                   name=f"gi{t}{hh}", bufs=2)
                    nc.sync.dma_start(out=gidx[:], in_=idx_in[t][:, ci0 * 8 : (ci0 + nslab) * 8])
                    gbuf = sb.tile([128, nslab, 128], BF16, tag=f"gb{t}{hh}",
                                   name=f"gb{t}{hh}", bufs=3 if hh == 0 else 2)
                    nc.gpsimd.dma_gather(
                        gbuf[:],
                        table[hh * HALFR : (hh + 1) * HALFR, :],
                        gidx[:],
                        nslab * 128,
                        nslab * 128,
                        128,
                        single_packet=False,
                    )
                    gb[(t, hh)] = (gbuf, ci0)

                gbs = [dict() for _ in WGROUPS]
                if l == 0:
                    # table0's half-0 finishes writing well before half-1:
                    # front-load the first two groups' half-0 gathers so the
                    # in-order Pool queue isn't blocked by a half-1 wait
                    for g in (0, 1):
                        for t in (0, 1):
                            emit_gather(t, 0, WGROUPS[g], gbs[g])
                    for g in (0, 1):
                        for t in (0, 1):
                            emit_gather(t, 1, WGROUPS[g], gbs[g])
                for g, wins in enumerate(WGROUPS):
                    gb = gbs[g]
                    if not gb:
                        for t in (0, 1):
                            for hh in (0, 1):
                                emit_gather(t, hh, wins, gb)
                    for w in wins:
                        agg_sb = []
                        for t in (0, 1):
                            nk = int(Ks[t][w, 0]) + int(Ks[t][w, 1])
                            pagg = ps.tile([128, 128], F32, tag="pagg", bufs=4)
                            ki = 0
                            for hh in (0, 1):
                                gbuf, ci0 = gb[(t, hh)]
                                slab0 = offs[t][(w, hh)] - ci0
                                for k in range(int(Ks[t][w, hh])):
                                    ci = offs[t][(w, hh)] + k
                                    s = sb.tile([128, 128], BF16, tag="s", bufs=72)
                                    nc.vector.tensor_scalar(
                                        out=s[:],
                                        in0=iota_sb[:],
                                        scalar1=dst_sb[t][:, ci : ci + 1],
                                        scalar2=v_sb[t][:, ci : ci + 1],
                                        op0=mybir.AluOpType.is_equal,
                                        op1=mybir.AluOpType.mult,
                                    )
                                    nc.tensor.matmul(pagg[:], lhsT=gbuf[:, slab0 + k, :], rhs=s[:],
                                                     start=(ki == 0), stop=(ki == nk - 1))
                                    ki += 1
                            a = sb.tile([128, 128], BF16, tag=f"agg{t}", bufs=3)
                            nc.scalar.activation(out=a[:], in_=pagg[:],
                                                 func=mybir.ActivationFunctionType.Copy)
                            agg_sb.append(a)
                        pf = ps.tile([128, 128], F32, tag="pf", bufs=1)
                        nc.tensor.matmul(pf[:], lhsT=w_sb[wa][:], rhs=agg_sb[0][:],
                                         start=True, stop=False)
                        nc.tensor.matmul(pf[:], lhsT=w_sb[wb][:], rhs=agg_sb[1][:],
                                         start=False, stop=False)
                        nc.tensor.matmul(pf[:], lhsT=w_sb[wl][:],
                                         rhs=src_hT[:, w * 128 : (w + 1) * 128],
                                         start=False, stop=True)
                        if l == 1:
                            o2 = sb.tile([128, 128], F32, tag="o2", bufs=3)
                            nc.scalar.activation(out=o2[:], in_=pf[:],
                                                 func=mybir.ActivationFunctionType.Relu,
                                                 bias=bias[:, :1], scale=1.0)
                            nc.sync.dma_start(out=out[:, w * 128 : (w + 1) * 128],
                                              in_=o2[:])
                        if l == 0:
                            nc.scalar.activation(out=h1T[:, w * 128 : (w + 1) * 128], in_=pf[:],
                                                 func=mybir.ActivationFunctionType.Relu,
                                                 bias=bias[:, :1], scale=1.0)
                            pt = ps.tile([128, 128], BF16, tag="ptr", bufs=1)
                            nc.tensor.transpose(pt[:], h1T[:, w * 128 : (w + 1) * 128],
                                                ident_sb[:])
                            hn = sb.tile([128, 128], BF16, tag="hn", bufs=2)
                            nc.scalar.activation(out=hn[:], in_=pt[:],
                                                 func=mybir.ActivationFunctionType.Copy)
                            nc.sync.dma_start(out=ag_in[w * 128 : (w + 1) * 128, :], in_=hn[:])
                if l == 0:
                    nc.gpsimd.collective_compute(
                        "AllGather",
                        mybir.AluOpType.bypass,
                        replica_groups=[list(range(NCORES))],
                        ins=[ag_in.opt()],
                        outs=[ag_out.opt()],
                    )

    nc.compile()
    return nc


def prepare(**inputs):
    """Build (or reuse) the compiled Bass module and the per-core input maps."""
    x = np.asarray(inputs["x"], np.float32)
    vmap = _layout(np.asarray(inputs["src_a"]), np.asarray(inputs["dst_a"]),
                   np.asarray(inputs["src_b"]), np.asarray(inputs["dst_b"]))
    prep_a = _prep_etype(np.asarray(inputs["src_a"]), np.asarray(inputs["dst_a"]), vmap)
    prep_b = _prep_etype(np.asarray(inputs["src_b"]), np.asarray(inputs["dst_b"]), vmap)
    K_a, off_a, nch_a, idx_a, dst_a, v_a = prep_a
    K_b, off_b, nch_b, idx_b, dst_b, v_b = prep_b

    key = (nch_a, nch_b, K_a.tobytes(), K_b.tobytes())
    if key not in _compiled:
        _compiled[key] = _build(K_a, off_a, nch_a, K_b, off_b, nch_b)
    nc = _compiled[key]

    x_pad = np.zeros((NCORES, SHARD, D), np.float32)
    x_pad.reshape(VN, D)[vmap] = x
    xnode = np.ascontiguousarray(x_pad.reshape(VN, D)).astype(BF)

    Wp_f = np.asarray(inputs["W_proj"], np.float32)
    # layer 1 aggregates raw x; W_proj is folded into its transform weights
    weights = {
        "W_proj": inputs["W_proj"],
        "W1_a": Wp_f @ np.asarray(inputs["W1_a"], np.float32),
        "W1_b": Wp_f @ np.asarray(inputs["W1_b"], np.float32),
        "loop1": inputs["loop1"], "W2_a": inputs["W2_a"], "W2_b": inputs["W2_b"],
        "loop2": inputs["loop2"],
    }
    w_np = {k: np.asarray(v, np.float32).astype(BF) for k, v in weights.items()}
    b_proj = np.asarray(inputs["b_proj"], np.float32)
    W1_a = np.asarray(inputs["W1_a"], np.float32)
    W1_b = np.asarray(inputs["W1_b"], np.float32)
    # table0 omits b_proj; its layer-1 contribution (b_proj @ W1_x per dst
    # row with in-degree > 0) is folded into bias1. Exact when b_proj == 0
    # (the given spec) or when no destination has zero in-degree.
    bias1_eff = (np.asarray(inputs["b1_a"], np.float32)
                 + np.asarray(inputs["b1_b"], np.float32)
                 + b_proj @ W1_a + b_proj @ W1_b)
    biases = {
        "bias_proj": b_proj.reshape(128, 1),
        "bias1": bias1_eff.reshape(128, 1),
        "bias2": (np.asarray(inputs["b2_a"], np.float32)
                  + np.asarray(inputs["b2_b"], np.float32)).reshape(128, 1),
    }
    iota = np.tile(np.arange(128, dtype=np.float32).astype(BF), (128, 1))
    ident = np.eye(128, dtype=np.float32).astype(BF)

    in_maps = []
    for c in range(NCORES):
        m = {
            "xT": np.ascontiguousarray(x_pad[c].T).astype(BF),
            "xnode": xnode,
            "iota": iota,
            "ident": ident,
            "idx_a": idx_a[c], "idx_b": idx_b[c],
            "dst_a": dst_a[c], "dst_b": dst_b[c],
            "v_a": v_a[c], "v_b": v_b[c],
        }
        m.update(w_np)
        m.update(biases)
        in_maps.append(m)
    return nc, in_maps


def kernel(**inputs):
    nc, in_maps = prepare(**inputs)
    res = run_bass_kernel_spmd(nc, in_maps, core_ids=list(range(NCORES)))
    globals()["_last_result"] = res
    vmap = _layout(np.asarray(inputs["src_a"]), np.asarray(inputs["dst_a"]),
                   np.asarray(inputs["src_b"]), np.asarray(inputs["dst_b"]))
    full_virt = np.concatenate(
        [np.asarray(res.results[c]["out"]).T for c in range(NCORES)], axis=0
    )
    return full_virt[vmap].astype(np.float32)


# revision 85
# speedup vs baseline: 1.0258x; 1.0168x over previous
"""2-layer GCN (2 edge types + self loop) on 8 TRN2 NeuronCores.

Sharding: nodes split contiguously across 8 cores (6250/core, padded to
6272 = 49 windows x 128 rows); edge lists partitioned by destination
owner, sorted by (dst window, src half); [128,128] weights replicated.

v2: aggregate-then-transform. Because GraphConv is linear,
  agg(A, h @ W) == agg(A, h) @ W,
so the table that must be shared each layer is h itself (shared by
BOTH edge types) instead of two per-etype m tables. Per dst window, the
two per-etype aggregates are built by selection-matrix matmuls
accumulating in PSUM (1/deg folded into the selection values), copied
to SBUF, then transformed by W_a/W_b plus the self-loop matmul in a
second PSUM group, with bias+ReLU fused into the PSUM->SBUF copy on the
scalar engine.

v3/v4: pushing the same linearity one level deeper,
  Ahat(x @ W_proj) == (Ahat x) @ W_proj,
so the layer-1 gather table is node-major x itself -- an input, uploaded
directly; no on-device table build and no layer-1 collective at all.
W_proj is folded into the layer-1 transform weights on the host
(W_proj @ W1_x in f32), and b_proj's aggregated contribution into the
layer-1 bias (exact whenever b_proj == 0 or no zero-degree nodes).
Only layer 2's table (h1, node-sharded) is built on device and
AllGathered. A host-side within-core permutation (_layout) shapes
per-(window, etype, src-half) edge counts toward just-under-multiples
of 128, cutting chunk padding to ~4%.
"""
import sys
import hashlib

sys.path.insert(0, "/opt/trn_rl_repo")

import numpy as np
import ml_dtypes

import concourse.bass as bass  # noqa: F401
import concourse.bacc as bacc
import concourse.mybir as mybir
import concourse.tile as tile
from concourse.bass_utils import run_bass_kernel_spmd

N = 50000
D = 128
NCORES = 8
LOCAL = 6250          # real rows per core
SHARD = 6272          # padded rows per core (49 windows of 128)
NW = 49               # dst windows per core
VN = SHARD * NCORES   # 50176 virtual node rows
HALFR = VN // 2       # 25088, int16-addressable half of the gathered table
WGROUPS = [list(range(i, min(i + 5, NW))) for i in range(0, NW, 5)]  # gather groups

F32 = mybir.dt.float32
BF16 = mybir.dt.bfloat16
I16 = mybir.dt.int16
BF = ml_dtypes.bfloat16

_compiled = {}
_prep_cache = {}

_layout_cache = {}


def _layout(src_a, dst_a, src_b, dst_b):
    """Within-core node->slot permutation that shapes per-(window, etype,
    src-half) edge counts toward just-under-multiples of 128, cutting chunk
    padding. Node->core stays contiguous (it defines the int16 half split),
    so the per-node components are fixed and the packing decouples per core.
    Returns vmap[node] -> virtual row (core*SHARD + slot)."""
    key = hashlib.sha1(src_a.tobytes() + dst_a.tobytes()
                       + src_b.tobytes() + dst_b.tobytes()).digest()
    if key in _layout_cache:
        return _layout_cache[key]
    core_of = np.arange(N, dtype=np.int64) // LOCAL
    comps = np.zeros((N, 4), np.int64)
    for ci, (src, dst) in enumerate(((src_a, dst_a), (src_b, dst_b))):
        hh = (core_of[src] >= NCORES // 2).astype(np.int64)
        np.add.at(comps, (dst, 2 * ci + hh), 1)
    # K template: a few heavy windows (K=9), rest light (K=8), same profile
    # for every core and component; bump if capacity can't cover the worst
    # per-core component total
    Ktmpl = np.where(np.arange(NW) % 4 == 1, 9, 8).astype(np.int64)
    worst = max(int(comps[r * LOCAL:(r + 1) * LOCAL].sum(axis=0).max())
                for r in range(NCORES))
    margin = 24
    while int((Ktmpl * 128 - margin).sum()) < worst:
        Ktmpl[int(np.argmin(Ktmpl))] += 1
    cap0 = Ktmpl * 128 - margin
    slot_of = np.zeros(N, np.int64)
    for r in range(NCORES):
        g = comps[r * LOCAL:(r + 1) * LOCAL]
        order = np.argsort(-g.sum(axis=1), kind="stable")
        cap = np.tile(cap0[:, None], (1, 4))
        slots_left = np.full(NW, 128, np.int64)
        slots_left[NW - 1] = LOCAL - 128 * (NW - 1)
        win_of = np.empty(LOCAL, np.int64)
        for d in order:
            head = (cap - g[d]).min(axis=1).astype(np.float64)
            head[slots_left <= 0] = -np.inf
            wsel = int(np.argmax(head))
            win_of[d] = wsel
            cap[wsel] -= g[d]
            slots_left[wsel] -= 1
        cnt = np.zeros(NW, np.int64)
        slot = np.empty(LOCAL, np.int64)
        for d in range(LOCAL):
            w = win_of[d]
            slot[d] = w * 128 + cnt[w]
            cnt[w] += 1
        slot_of[r * LOCAL:(r + 1) * LOCAL] = slot
    vmap = core_of * SHARD + slot_of
    _layout_cache[key] = vmap
    return vmap



def _prep_etype(src, dst, vmap):
    """Per-edge-type host prep. Returns (K[w][h] chunk table,
    per-core wrapped idx / dst_rel / v arrays laid out in canonical
    chunk order: for g, for h, for w in g, for k)."""
    key = hashlib.sha1(src.tobytes() + dst.tobytes() + vmap.tobytes()).digest()
    if key in _prep_cache:
        return _prep_cache[key]
    deg = np.bincount(dst, minlength=N).astype(np.float32)
    v_edge = (1.0 / np.maximum(deg, 1.0))[dst].astype(np.float32)
    dvid = vmap[dst]
    r = dvid // SHARD
    w = (dvid % SHARD) // 128
    wloc = (dvid % 128).astype(np.float32)
    svid = vmap[src]
    h = svid // HALFR
    i16 = (svid % HALFR).astype(np.int16)

    key2 = (r * NW + w) * 2 + h
    order = np.argsort(key2, kind="stable")
    counts = np.bincount(key2, minlength=NCORES * NW * 2).reshape(NCORES, NW, 2)
    flat = counts.reshape(-1)
    fs = np.concatenate([[0], np.cumsum(flat)[:-1]])
    starts = fs.reshape(NCORES, NW, 2)

    K = np.maximum(1, (counts.max(axis=0) + 127) // 128)  # [NW, 2]

    # canonical chunk order
    chunk_off = {}
    nch = 0
    for g in WGROUPS:
        for hh in (0, 1):
            for ww in g:
                chunk_off[(ww, hh)] = nch
                nch += int(K[ww, hh])

    i16_s = i16[order]
    wloc_s = wloc[order]
    v_s = v_edge[order]

    idx_all, dst_all, v_all = [], [], []
    for rr in range(NCORES):
        idx_pad = np.zeros(nch * 128, np.int16)
        dst_pad = np.full(nch * 128, -1.0, np.float32)
        v_pad = np.ones(nch * 128, np.float32)
        for ww in range(NW):
            for hh in (0, 1):
                s0 = starts[rr, ww, hh]
                c = counts[rr, ww, hh]
                o = chunk_off[(ww, hh)] * 128
                idx_pad[o : o + c] = i16_s[s0 : s0 + c]
                dst_pad[o : o + c] = wloc_s[s0 : s0 + c]
                v_pad[o : o + c] = v_s[s0 : s0 + c]
        wrapped = np.tile(idx_pad.reshape(-1, 16).T, (8, 1))  # [128, nch*8]
        idx_all.append(np.ascontiguousarray(wrapped))
        dst_all.append(np.ascontiguousarray(dst_pad.reshape(nch, 128).T))
        v_all.append(np.ascontiguousarray(v_pad.reshape(nch, 128).T))
    res = (K, chunk_off, nch, idx_all, dst_all, v_all)
    _prep_cache[key] = res
    return res


def _build(K_a, off_a, nch_a, K_b, off_b, nch_b):
    nc = bacc.Bacc("TRN2", target_bir_lowering=False, debug=False)

    xT_in = nc.dram_tensor("xT", [128, SHARD], BF16, kind="ExternalInput")
    xnode_in = nc.dram_tensor("xnode", [VN, 128], BF16, kind="ExternalInput")
    w_names = ["W_proj", "W1_a", "W1_b", "loop1", "W2_a", "W2_b", "loop2"]
    w_in = {n: nc.dram_tensor(n, [128, 128], BF16, kind="ExternalInput") for n in w_names}
    b_names = ["bias_proj", "bias1", "bias2"]
    b_in = {n: nc.dram_tensor(n, [128, 1], F32, kind="ExternalInput") for n in b_names}
    iota_in = nc.dram_tensor("iota", [128, 128], BF16, kind="ExternalInput")
    ident_in = nc.dram_tensor("ident", [128, 128], BF16, kind="ExternalInput")
    idx_in = [
        nc.dram_tensor("idx_a", [128, nch_a * 8], I16, kind="ExternalInput"),
        nc.dram_tensor("idx_b", [128, nch_b * 8], I16, kind="ExternalInput"),
    ]
    dst_in = [
        nc.dram_tensor("dst_a", [128, nch_a], F32, kind="ExternalInput"),
        nc.dram_tensor("dst_b", [128, nch_b], F32, kind="ExternalInput"),
    ]
    v_in = [
        nc.dram_tensor("v_a", [128, nch_a], F32, kind="ExternalInput"),
        nc.dram_tensor("v_b", [128, nch_b], F32, kind="ExternalInput"),
    ]
    out = nc.dram_tensor("out", [128, SHARD], F32, kind="ExternalOutput")

    Ks = [K_a, K_b]
    offs = [off_a, off_b]

    with tile.TileContext(nc) as tc:
        with (
            tc.tile_pool(name="sbuf", bufs=1) as sb,
            tc.tile_pool(name="psum", bufs=1, space="PSUM") as ps,
            tc.tile_pool(name="dram", bufs=1, space="DRAM") as dr,
        ):
            # ---- constants / persistent buffers
            # all constants load via the idle ACT queue; SP's queue is then
            # purely gather-index loads, so the first gather fires immediately
            w_sb = {}
            for n in w_names:
                w_sb[n] = sb.tile([128, 128], BF16, tag=f"w_{n}", name=f"w_{n}")
                nc.scalar.dma_start(out=w_sb[n][:], in_=w_in[n][:])
            b_sb = {}
            for n in b_names:
                b_sb[n] = sb.tile([128, 1], F32, tag=f"b_{n}", name=f"b_{n}")
                nc.scalar.dma_start(out=b_sb[n][:], in_=b_in[n][:])
            iota_sb = sb.tile([128, 128], BF16, tag="iota")
            nc.scalar.dma_start(out=iota_sb[:], in_=iota_in[:])
            ident_sb = sb.tile([128, 128], BF16, tag="ident")
            nc.scalar.dma_start(out=ident_sb[:], in_=ident_in[:])

            # xT / dst / v go through the otherwise-idle ACT queue so the
            # SP queue reaches the first gather's index loads immediately
            xT = sb.tile([128, SHARD], BF16, tag="hstate", bufs=2)
            nc.scalar.dma_start(out=xT[:], in_=xT_in[:])
            dst_sb = []
            v_sb = []
            for t in (0, 1):
                d = sb.tile([128, [nch_a, nch_b][t]], F32, tag=f"dst{t}", name=f"dst{t}")
                nc.scalar.dma_start(out=d[:], in_=dst_in[t][:])
                dst_sb.append(d)
                vv = sb.tile([128, [nch_a, nch_b][t]], F32, tag=f"v{t}", name=f"v{t}")
                nc.scalar.dma_start(out=vv[:], in_=v_in[t][:])
                v_sb.append(vv)

            hT = sb.tile([128, SHARD], BF16, tag="hstate", bufs=2)
            h1T = sb.tile([128, SHARD], BF16, tag="hstate", bufs=2)

            # layer-1 gather table: by linearity Ahat(x@W_proj) ==
            # (Ahat x)@W_proj, so the table is node-major x itself (an
            # input); W_proj is folded into the layer-1 transform weights
            # on the host. Only layer 2's table (h1) needs building+AllGather.
            table0 = xnode_in
            ag_in = dr.tile([SHARD, 128], BF16, tag="agi1", name="agi1")
            ag_out = dr.tile([VN, 128], BF16, tag="ago1", name="ago1", addr_space="Shared")

            def col_chunks(total, step):
                o = 0
                while o < total:
                    yield o, min(step, total - o)
                    o += step

            # ---- phase P: hT = (x @ W_proj + b_proj)^T  (feature-major,
            # local; feeds the self-loop terms)
            for o, n in col_chunks(SHARD, 512):
                p = ps.tile([128, 512], F32, tag="pdense", bufs=2)
                nc.tensor.matmul(p[:, :n], lhsT=w_sb["W_proj"][:], rhs=xT[:, o : o + n],
                                 start=True, stop=True)
                nc.vector.tensor_scalar_add(hT[:, o : o + n], p[:, :n], b_sb["bias_proj"][:, :1])

            # ---- layers
            for l in (0, 1):
                src_hT = hT if l == 0 else h1T
                wa, wb, wl = (("W1_a", "W1_b", "loop1") if l == 0 else ("W2_a", "W2_b", "loop2"))
                bias = b_sb["bias1"] if l == 0 else b_sb["bias2"]
                table = table0 if l == 0 else ag_out

                def emit_gather(t, hh, wins, gb):
                    nslab = sum(int(Ks[t][w, hh]) for w in wins)
                    ci0 = offs[t][(wins[0], hh)]
                    gidx = sb.tile([128, nslab * 8], I16, tag=f"gi{t}{hh}",
                                   name=f"gi{t}{hh}", bufs=2)
                    nc.sync.dma_start(out=gidx[:], in_=idx_in[t][:, ci0 * 8 : (ci0 + nslab) * 8])
                    gbuf = sb.tile([128, nslab, 128], BF16, tag=f"gb{t}{hh}",
                                   name=f"gb{t}{hh}", bufs=3 if hh == 0 else 2)
                    nc.gpsimd.dma_gather(
                        gbuf[:],
                        table[hh * HALFR : (hh + 1) * HALFR, :],
                        gidx[:],
                        nslab * 128,
                        nslab * 128,
                        128,
                        single_packet=False,
                    )
                    gb[(t, hh)] = (gbuf, ci0)

                gbs = [dict() for _ in WGROUPS]
                if l == 0:
                    # table0's half-0 finishes writing well before half-1:
                    # front-load the first two groups' half-0 gathers so the
                    # in-order Pool queue isn't blocked by a half-1 wait
                    for g in (0, 1):
                        for t in (0, 1):
                            emit_gather(t, 0, WGROUPS[g], gbs[g])
                    for g in (0, 1):
                        for t in (0, 1):
                            emit_gather(t, 1, WGROUPS[g], gbs[g])
                for g, wins in enumerate(WGROUPS):
                    gb = gbs[g]
                    if not gb:
                        for t in (0, 1):
                            for hh in (0, 1):
                                emit_gather(t, hh, wins, gb)
                    for w in wins:
                        agg_sb = []
                        for t in (0, 1):
                            nk = int(Ks[t][w, 0]) + int(Ks[t][w, 1])
                            pagg = ps.tile([128, 128], F32, tag="pagg", bufs=4)
                            ki = 0
                            for hh in (0, 1):
                                gbuf, ci0 = gb[(t, hh)]
                                slab0 = offs[t][(w, hh)] - ci0
                                for k in range(int(Ks[t][w, hh])):
                                    ci = offs[t][(w, hh)] + k
                                    s = sb.tile([128, 128], BF16, tag="s", bufs=72)
                                    nc.vector.tensor_scalar(
                                        out=s[:],
                                        in0=iota_sb[:],
                                        scalar1=dst_sb[t][:, ci : ci + 1],
                                        scalar2=v_sb[t][:, ci : ci + 1],
                                        op0=mybir.AluOpType.is_equal,
                                        op1=mybir.AluOpType.mult,
                                    )
                                    nc.tensor.matmul(pagg[:], lhsT=gbuf[:, slab0 + k, :], rhs=s[:],
                                                     start=(ki == 0), stop=(ki == nk - 1))
                                    ki += 1
                            a = sb.tile([128, 128], BF16, tag=f"agg{t}", bufs=3)
                            nc.scalar.activation(out=a[:], in_=pagg[:],
                                                 func=mybir.ActivationFunctionType.Copy)
                            agg_sb.append(a)
                        pf = ps.tile([128, 128], F32, tag="pf", bufs=1)
                        nc.tensor.matmul(pf[:], lhsT=w_sb[wa][:], rhs=agg_sb[0][:],
                                         start=True, stop=False)
                        nc.tensor.matmul(pf[:], lhsT=w_sb[wb][:], rhs=agg_sb[1][:],
                                         start=False, stop=False)
                        nc.tensor.matmul(pf[:], lhsT=w_sb[wl][:],
                                         rhs=src_hT[:, w * 128 : (w + 1) * 128],
                                         start=False, stop=True)
                        if l == 1:
                            o2 = sb.tile([128, 128], F32, tag="o2", bufs=3)
                            nc.scalar.activation(out=o2[:], in_=pf[:],
                                                 func=mybir.ActivationFunctionType.Relu,
                                                 bias=bias[:, :1], scale=1.0)
                            nc.sync.dma_start(out=out[:, w * 128 : (w + 1) * 128],
                                              in_=o2[:])
                        if l == 0:
                            nc.scalar.activation(out=h1T[:, w * 128 : (w + 1) * 128], in_=pf[:],
                                                 func=mybir.ActivationFunctionType.Relu,
                                                 bias=bias[:, :1], scale=1.0)
                            pt = ps.tile([128, 128], BF16, tag="ptr", bufs=1)
                            nc.tensor.transpose(pt[:], h1T[:, w * 128 : (w + 1) * 128],
                                                ident_sb[:])
                            hn = sb.tile([128, 128], BF16, tag="hn", bufs=2)
                            nc.scalar.activation(out=hn[:], in_=pt[:],
                                                 func=mybir.ActivationFunctionType.Copy)
                            nc.sync.dma_start(out=ag_in[w * 128 : (w + 1) * 128, :], in_=hn[:])
                if l == 0:
                    nc.gpsimd.collective_compute(
                        "AllGather",
                        mybir.AluOpType.bypass,
                        replica_groups=[list(range(NCORES))],
                        ins=[ag_in.opt()],
                        outs=[ag_out.opt()],
                    )

    nc.compile()
    return nc


def prepare(**inputs):
    """Build (or reuse) the compiled Bass module and the per-core input maps."""
    x = np.asarray(inputs["x"], np.float32)
    vmap = _layout(np.asarray(inputs["src_a"]), np.asarray(inputs["dst_a"]),
                   np.asarray(inputs["src_b"]), np.asarray(inputs["dst_b"]))
    prep_a = _prep_etype(np.asarray(inputs["src_a"]), np.asarray(inputs["dst_a"]), vmap)
    prep_b = _prep_etype(np.asarray(inputs["src_b"]), np.asarray(inputs["dst_b"]), vmap)
    K_a, off_a, nch_a, idx_a, dst_a, v_a = prep_a
    K_b, off_b, nch_b, idx_b, dst_b, v_b = prep_b

    key = (nch_a, nch_b, K_a.tobytes(), K_b.tobytes())
    if key not in _compiled:
        _compiled[key] = _build(K_a, off_a, nch_a, K_b, off_b, nch_b)
    nc = _compiled[key]

    x_pad = np.zeros((NCORES, SHARD, D), np.float32)
    x_pad.reshape(VN, D)[vmap] = x
    xnode = np.ascontiguousarray(x_pad.reshape(VN, D)).astype(BF)

    Wp_f = np.asarray(inputs["W_proj"], np.float32)
    # layer 1 aggregates raw x; W_proj is folded into its transform weights
    weights = {
        "W_proj": inputs["W_proj"],
        "W1_a": Wp_f @ np.asarray(inputs["W1_a"], np.float32),
        "W1_b": Wp_f @ np.asarray(inputs["W1_b"], np.float32),
        "loop1": inputs["loop1"], "W2_a": inputs["W2_a"], "W2_b": inputs["W2_b"],
        "loop2": inputs["loop2"],
    }
    w_np = {k: np.asarray(v, np.float32).astype(BF) for k, v in weights.items()}
    b_proj = np.asarray(inputs["b_proj"], np.float32)
    W1_a = np.asarray(inputs["W1_a"], np.float32)
    W1_b = np.asarray(inputs["W1_b"], np.float32)
    # table0 omits b_proj; its layer-1 contribution (b_proj @ W1_x per dst
    # row with in-degree > 0) is folded into bias1. Exact when b_proj == 0
    # (the given spec) or when no destination has zero in-degree.
    bias1_eff = (np.asarray(inputs["b1_a"], np.float32)
                 + np.asarray(inputs["b1_b"], np.float32)
                 + b_proj @ W1_a + b_proj @ W1_b)
    biases = {
        "bias_proj": b_proj.reshape(128, 1),
        "bias1": bias1_eff.reshape(128, 1),
        "bias2": (np.asarray(inputs["b2_a"], np.float32)
                  + np.asarray(inputs["b2_b"], np.float32)).reshape(128, 1),
    }
    iota = np.tile(np.arange(128, dtype=np.float32).astype(BF), (128, 1))
    ident = np.eye(128, dtype=np.float32).astype(BF)

    in_maps = []
    for c in range(NCORES):
        m = {
            "xT": np.ascontiguousarray(x_pad[c].T).astype(BF),
            "xnode": xnode,
            "iota": iota,
            "ident": ident,
            "idx_a": idx_a[c], "idx_b": idx_b[c],
            "dst_a": dst_a[c], "dst_b": dst_b[c],
            "v_a": v_a[c], "v_b": v_b[c],
        }
        m.update(w_np)
        m.update(biases)
        in_maps.append(m)
    return nc, in_maps


def kernel(**inputs):
    nc, in_maps = prepare(**inputs)
    res = run_bass_kernel_spmd(nc, in_maps, core_ids=list(range(NCORES)))
    globals()["_last_result"] = res
    vmap = _layout(np.asarray(inputs["src_a"]), np.asarray(inputs["dst_a"]),
                   np.asarray(inputs["src_b"]), np.asarray(inputs["dst_b"]))
    full_virt = np.concatenate(
        [np.asarray(res.results[c]["out"]).T for c in range(NCORES)], axis=0
    )
    return full_virt[vmap].astype(np.float32)
